# revision 19
# baseline (speedup 1.0000x reference)
"""Trainium2 Bass kernel for nn_C2BM_30537217474758 (gnn_message_passing).

Concept-bottleneck model:
  x_enc = lrelu(x @ W_enc + b_enc)                         [B, 1024]
  vals  = lrelu(einsum('bi,rio->bro', x_enc, Wv) + bv)     [B, 8, 256]
  p_root = softmax(einsum('bro,roc->brc', vals, Ws) + bs)  [B, 8, 4]
  p_root = intervene(p_root, c[:, :8], ii[:, :8])
  h     = lrelu(einsum('bp,nph->bnh', p_root.flat, W1c) + b1c)
  p_mid = softmax(einsum('bnh,nhc->bnc', h, W2c) + b2c); intervene
  y     = softmax(lrelu(p_mid.flat @ W1y + b1y) @ W2y + b2y)
  out   = concat([p_root, p_mid, y[:, None]], axis=1)      [B, 17, 4]

Strategy: pure data-parallel over 8 NeuronCores (batch shard 1024/core),
weights replicated.  The two large GEMMs (encoder and value-embedding) run
as fp8e4m3 DoubleRow matmuls (256-row virtual contraction, ~1.5x bf16 PE
rate) with fp32 PSUM accumulation; inputs are pre-scaled on the host to
keep fp8 operands in the normal range and the scales are unwound in the
PSUM drains.  x is transposed AND pre-quantized on the host, so no
on-chip transposes of x are needed at all.  All leaky-relus run on the
DVE (max(x, .01x)); the ACT engine only ever evaluates Identity/Exp so
its function-table is loaded exactly once.  Small weights are packed into
three canvas tensors loaded in single DMAs before the big streams, and
x/W stream in interleaved 256KB chunks so the encoder starts ~4us in.
Dummy matmuls at the head keep the PE HAM un-throttled through the DMA
ramp.  Everything downstream of the big GEMMs (softmax, intervention,
propagator MLPs) is bf16/fp32 exactly like the reference.
"""

import os
import sys

try:
    import concourse  # noqa: F401
except ImportError:
    sys.path.insert(0, "/opt/trn_rl_repo")

import numpy as np
import ml_dtypes

import concourse.bacc as bacc
import concourse.tile as tile
from concourse import mybir

# ---------------- problem constants (hardcoded per contract) ----------------
B, D_IN, D_H = 8192, 2048, 1024
N_ROOT, N_MID, CARD, CHS = 8, 8, 4, 64
OV = CARD * CHS           # 256  value-embedding width per root
P_IN = N_ROOT * CARD      # 32
P_HID = 2 * P_IN          # 64
N_CORES = 8
BSH = B // N_CORES        # 1024 batch rows per core
OUTW = 17 * CARD          # 68 output cols per row
KJ_IN = D_IN // 256       # 8 DoubleRow k-chunks for encoder
KJ_H = D_H // 256         # 4 DoubleRow k-chunks for Wv

F32 = mybir.dt.float32
I32 = mybir.dt.int32
BF16 = mybir.dt.bfloat16
FP8 = mybir.dt.float8e4
AF = mybir.ActivationFunctionType
ALU = mybir.AluOpType
AX = mybir.AxisListType
DR = mybir.MatmulPerfMode.DoubleRow

USE_FP8 = os.environ.get("BASS_FP8", "1") != "0"
MDT = FP8 if USE_FP8 else BF16
MDT_NP = ml_dtypes.float8_e4m3 if USE_FP8 else ml_dtypes.bfloat16
LRELU_ALPHA = 0.01
N_WARM = int(os.environ.get("BASS_WARM", "28"))

# fp8 pre-scales (host) and their unwind factors (on-chip activation scale)
SX = 16.0 if USE_FP8 else 1.0    # x
SW = 512.0 if USE_FP8 else 1.0   # W_enc
SH = 32.0 if USE_FP8 else 1.0    # x_enc (re-quantized fp8 out of encoder)
SV = 512.0 if USE_FP8 else 1.0   # Wv
SCALE1 = SH / (SX * SW)          # psum1 -> SH * x_enc
SCALE2 = 1.0 / (SH * SV)         # psum2 -> vals (unscaled)

# bf16 canvas column layout
C_WS = 0            # [128, 2, 32]  scorer Ws
C_W1C = 64          # [33, 4, 128]  mid W1c pairs + b1c ones-row
C_W2C = 576         # [128, 4, 8]   mid W2c block-pairs
C_W1Y = 608         # [33, 64]      task W1y + b1y row
C_W2Y = 672         # [65, 4]       task W2y + b2y row
C_BSR = 676         # [1, 32]       root scorer bias row
C_B2CR = 708        # [1, 32]       mid bias row
C_IDENT = 740       # [128, 128]    identity for PE transposes
SMB_COLS = 868
# f32 canvas
C_BENC = 0          # [128, 8]   SH * b_enc, [p, ht]
C_BV = 8            # [128, 16]  bv, [p, (r, ot)]
C_IOTA = 24         # [128, 32]  tiled iota(4) for one-hot build
F32_COLS = 56


def build_program():
    """Emit the per-core Bass program (identical on all 8 cores)."""
    nc = bacc.Bacc("TRN2", target_bir_lowering=False, debug=False,
                   num_devices=N_CORES)

    # ------------- DRAM I/O -------------
    xt_d = nc.dram_tensor("xt", [128, KJ_IN * 2 * BSH], MDT,
                          kind="ExternalInput")
    wenc_d = nc.dram_tensor("wenc", [128, KJ_IN * 2 * D_H], MDT,
                            kind="ExternalInput")
    wv_d = nc.dram_tensor("wv", [128, KJ_H * 2 * N_ROOT * OV], MDT,
                          kind="ExternalInput")
    smb_d = nc.dram_tensor("smb", [128, SMB_COLS], BF16, kind="ExternalInput")
    f32c_d = nc.dram_tensor("f32c", [128, F32_COLS], F32, kind="ExternalInput")
    lm_d = nc.dram_tensor("lm", [128, 2 * 8 * 17], I32, kind="ExternalInput")
    out_d = nc.dram_tensor("out", [BSH, OUTW], F32, kind="ExternalOutput")

    with tile.TileContext(nc) as tc:
        with (
            tc.tile_pool(name="persist", bufs=1) as persist,
            tc.tile_pool(name="vals", bufs=2) as vals_pool,
            tc.tile_pool(name="stage", bufs=3) as stage_pool,
            tc.tile_pool(name="tmp32", bufs=4) as tmp32_pool,
            tc.tile_pool(name="tmp8", bufs=6) as tmp8_pool,
            tc.tile_pool(name="ps_mm", bufs=4, space="PSUM") as ps_mm,
            tc.tile_pool(name="ps_lg", bufs=2, space="PSUM") as ps_lg,
            tc.tile_pool(name="ps_tr", bufs=2, space="PSUM") as ps_tr,
        ):
            # ---------------- persistent SBUF ----------------
            smb_sb = persist.tile([128, SMB_COLS], BF16)
            f32c_sb = persist.tile([128, F32_COLS], F32)
            lm_sb = persist.tile([128, 2 * 8 * 17], I32)
            xt_sb = persist.tile([128, KJ_IN, 2, BSH], MDT)
            wenc_sb = persist.tile([128, KJ_IN, 2, D_H], MDT)
            wv_sb = persist.tile([128, KJ_H, 2, N_ROOT * OV], MDT)
            xenc_sb = persist.tile([128, KJ_H, 2, BSH], MDT)
            prT_sb = persist.tile([P_IN + 1, BSH], BF16)
            pmT_sb = persist.tile([P_IN + 1, BSH], BF16)
            hyT_sb = persist.tile([P_HID + 1, BSH], BF16)
            hT_sb = persist.tile([128, 4, BSH], BF16)
            ones_sb = persist.tile([1, 128], BF16)
            spam_sb = persist.tile([128, 512], BF16)
            dume_sb = persist.tile([128, 8], F32)
            osb_gs = [persist.tile([128, 4 * OUTW], F32, name=f"osbg{i}")
                      for i in range(2)]

            ws_v = smb_sb[:, C_WS:C_WS + 64].rearrange(
                "p (k c) -> p k c", c=32)
            w1c_v = smb_sb[0:P_IN + 1, C_W1C:C_W1C + 512].rearrange(
                "p (q m) -> p q m", m=128)
            w2c_v = smb_sb[:, C_W2C:C_W2C + 32].rearrange(
                "p (q c) -> p q c", c=8)
            w1y_v = smb_sb[0:P_IN + 1, C_W1Y:C_W1Y + P_HID]
            w2y_v = smb_sb[0:P_HID + 1, C_W2Y:C_W2Y + CARD]
            bsr_v = smb_sb[0:1, C_BSR:C_BSR + 32]
            b2cr_v = smb_sb[0:1, C_B2CR:C_B2CR + 32]
            ident_v = smb_sb[:, C_IDENT:C_IDENT + 128]
            benc_v = f32c_sb[:, C_BENC:C_BENC + 8]
            bv_v = f32c_sb[:, C_BV:C_BV + 16]
            iota_v = f32c_sb[:, C_IOTA:C_IOTA + 32]
            lab_v = lm_sb[:, 0:136].rearrange("p (t k) -> p t k", k=17)
            msk_v = lm_sb[:, 136:272].rearrange("p (t k) -> p t k", k=17)

            # ------------- prologue: memsets + ACT table pin -------------
            nc.vector.memset(ones_sb, 1.0)
            nc.vector.memset(spam_sb, 0.0)
            nc.vector.memset(dume_sb, 0.0)
            nc.vector.memset(hyT_sb[P_HID:P_HID + 1, :], 1.0)
            dume2 = persist.tile([128, 8], F32, name="dume2")
            # first ACT instruction is an Exp -> its table set stays
            # resident for the whole kernel (Identity rides along free)
            nc.scalar.activation(dume2, dume_sb, AF.Exp)

            # ------------- DMA schedule: smalls first, then streams ------
            nc.sync.dma_start(out=smb_sb, in_=smb_d.ap())
            nc.sync.dma_start(out=f32c_sb, in_=f32c_d.ap())
            nc.sync.dma_start(out=lm_sb, in_=lm_d.ap())
            wenc_r = wenc_d.ap().rearrange("p (j i k) -> p j i k",
                                           j=KJ_IN, i=2)
            xt_r = xt_d.ap().rearrange("p (j i k) -> p j i k", j=KJ_IN, i=2)
            wv_r = wv_d.ap().rearrange("p (j i k) -> p j i k", j=KJ_H, i=2)
            for j in range(KJ_IN):
                nc.sync.dma_start(out=wenc_sb[:, j], in_=wenc_r[:, j])
                nc.sync.dma_start(out=xt_sb[:, j], in_=xt_r[:, j])
            # wv in 4 column chunks (2 roots each) so vals r0/r1 can start
            # before the whole tensor lands
            for c in range(4):
                nc.sync.dma_start(out=wv_sb[:, :, :, 512 * c:512 * (c + 1)],
                                  in_=wv_r[:, :, :, 512 * c:512 * (c + 1)])

            # ------------- PE warm-up spam (during DMA ramp) -------------
            def warm(n, name):
                ps = ps_mm.tile([128, 128], F32, tag="mm", name=name)
                for _ in range(n):
                    nc.tensor.matmul(ps, spam_sb[:, 0:128],
                                     spam_sb[:, 0:128],
                                     start=True, stop=True,
                                     skip_group_check=True)

            warm(N_WARM, "warm0")

            # ------------- helpers -------------
            def mm_pair(ps, w4, x4, start, stop):
                """One 256-deep contraction step: DoubleRow (fp8) or two
                bf16 matmuls. w4/x4 are [128, 2, M/N] slices."""
                if USE_FP8:
                    nc.tensor.matmul(ps, w4, x4, start=start, stop=stop,
                                     perf_mode=DR)
                else:
                    nc.tensor.matmul(ps, w4[:, 0], x4[:, 0],
                                     start=start, stop=False)
                    nc.tensor.matmul(ps, w4[:, 1], x4[:, 1],
                                     start=False, stop=stop)

            def lrelu(out, in_):
                nc.vector.scalar_tensor_tensor(out, in_, LRELU_ALPHA, in_,
                                               op0=ALU.mult, op1=ALU.max)

            def lrelu_psum(out, ps, nrows=128):
                """DVE can read only one PSUM operand; stage via ACT Copy."""
                st = stage_pool.tile([nrows, 512], BF16, tag="encst")
                nc.scalar.activation(st, ps, AF.Copy)
                lrelu(out, st)

            def drain_enc(ht, ps, g):
                st = stage_pool.tile([128, 512], BF16, tag="encst")
                nc.scalar.activation(st, ps, AF.Identity,
                                     bias=benc_v[:, ht:ht + 1], scale=SCALE1)
                lrelu(xenc_sb[:, ht // 2, ht % 2, g * 512:(g + 1) * 512], st)

            # ------------- encoder h0: kt-outer, 6+2 banks -------
            # Pass A uses 4 ps_mm banks plus the (still idle) ps_lg banks
            # so six output tiles track the x/W DMA chunk cadence; pass B
            # (2 tiles, ps_tr banks) runs while pass A drains.
            for hts, pools in ([0, 1, 2, 3, 4, 5], (ps_mm, ps_mm, ps_mm,
                                                    ps_mm, ps_lg, ps_lg)), \
                              ([6, 7], (ps_tr, ps_tr)):
                tagof = {id(ps_mm): "mm", id(ps_lg): "lg", id(ps_tr): "ptr"}
                pss = [pool.tile([128, 512], F32, tag=tagof[id(pool)],
                                 name=f"encp{ht}")
                       for ht, pool in zip(hts, pools)]
                for j in range(KJ_IN):
                    for i, ht in enumerate(hts):
                        mm_pair(pss[i],
                                wenc_sb[:, j, :, ht * 128:(ht + 1) * 128],
                                xt_sb[:, j, :, 0:512],
                                start=(j == 0), stop=(j == KJ_IN - 1))
                for i, ht in enumerate(hts):
                    drain_enc(ht, pss[i], 0)

            # ------------- intervention one-hots / masks (DVE) -----------
            oh_t, m_t = {}, {}

            def pview(t):
                return (t.rearrange("p (b k) -> p b k", k=P_HID)[:, :, 0:32]
                        .rearrange("p b (g c) -> p b g c", c=CARD))

            def make_ohm(g, lv):
                labf = tmp8_pool.tile([128, 32], F32, tag="labf")
                nc.vector.tensor_copy(
                    labf.rearrange("p (b g) -> p b g", b=4),
                    lab_v[:, 4 * g:4 * g + 4, lv * 8:lv * 8 + 8])
                oh = persist.tile([128, 256], BF16, name=f"oh{g}{lv}")
                nc.vector.tensor_tensor(
                    pview(oh),
                    labf.rearrange("p (b g) -> p b g", b=4)
                    .unsqueeze(3).broadcast_to([128, 4, 8, CARD]),
                    iota_v.rearrange("p (g c) -> p g c", c=CARD)
                    .unsqueeze(1).broadcast_to([128, 4, 8, CARD]),
                    op=ALU.is_equal)
                m = persist.tile([128, 256], I32, name=f"m{g}{lv}")
                nc.vector.tensor_copy(
                    pview(m),
                    msk_v[:, 4 * g:4 * g + 4, lv * 8:lv * 8 + 8]
                    .unsqueeze(3).broadcast_to([128, 4, 8, CARD]))
                oh_t[(g, lv)] = oh
                m_t[(g, lv)] = m

            for g in range(2):
                for lv in range(2):
                    make_ohm(g, lv)

            # ------------- per-root value GEMM + scorer (one half) -------
            def vals_scorer_half(g, lg, extra_pe=None):
                for bti in range(4):
                    nc.tensor.matmul(
                        lg[:, bti * 32:(bti + 1) * 32], ones_sb, bsr_v,
                        start=True, stop=False, skip_group_check=True)
                for r in range(N_ROOT):
                    vals_sb = vals_pool.tile([128, 2, 512], BF16, tag="vals")
                    for ot in range(2):
                        ps = ps_mm.tile([128, 512], F32, tag="mm")
                        for jh in range(KJ_H):
                            mm_pair(ps,
                                    wv_sb[:, jh, :,
                                          r * OV + ot * 128:
                                          r * OV + (ot + 1) * 128],
                                    xenc_sb[:, jh, :, g * 512:(g + 1) * 512],
                                    start=(jh == 0), stop=(jh == KJ_H - 1))
                        st = stage_pool.tile([128, 512], BF16, tag="encst")
                        nc.scalar.activation(
                            st, ps, AF.Identity,
                            bias=bv_v[:, 2 * r + ot:2 * r + ot + 1],
                            scale=SCALE2)
                        lrelu(vals_sb[:, ot], st)
                    for bti in range(4):
                        dst = lg[:, bti * 32 + r * 4:bti * 32 + r * 4 + 4]
                        for kt in range(2):
                            nc.tensor.matmul(
                                dst, vals_sb[:, kt, bti * 128:(bti + 1) * 128],
                                ws_v[:, kt, r * 4:(r + 1) * 4],
                                start=False, stop=(kt == 1),
                                skip_group_check=True)
                    if extra_pe and r in extra_pe:
                        extra_pe[r]()

            # ---------------- tail stages for one half ----------------
            def softmax_chain(g, lg, lv):
                """softmax + intervention on [128, 4bt x 32] logits ->
                pfin [128, 4bt x (32 probs | one | pad)] bf16."""
                e = tmp32_pool.tile([128, 128], F32, tag="e")
                nc.scalar.activation(e, lg, AF.Exp)
                s = tmp8_pool.tile([128, 32], F32, tag="s")
                nc.vector.reduce_sum(s, e.rearrange("p (x c) -> p x c",
                                                    c=CARD), axis=AX.X)
                rcp = tmp8_pool.tile([128, 32], F32, tag="rcp")
                nc.vector.reciprocal(rcp, s)
                pfin = tmp32_pool.tile([128, 256], BF16, tag="pfin")
                nc.vector.memset(
                    pfin.rearrange("p (b k) -> p b k", k=P_HID)[:, :, 32:33],
                    1.0)
                nc.vector.tensor_tensor(
                    pview(pfin),
                    e.rearrange("p (b g c) -> p b g c", b=4, c=CARD),
                    rcp.rearrange("p (b g) -> p b g", b=4)
                    .unsqueeze(3).broadcast_to([128, 4, 8, CARD]),
                    op=ALU.mult)
                nc.vector.copy_predicated(pview(pfin), pview(m_t[(g, lv)]),
                                          pview(oh_t[(g, lv)]))
                return pfin

            def osb_view(g, lo, hi):
                return (osb_gs[g].rearrange("p (b k) -> p b k", k=OUTW)
                        [:, :, lo * 4:hi * 4]
                        .rearrange("p b (gg c) -> p b gg c", c=CARD))

            def osb_store(g, pfin, lv):
                nc.vector.tensor_copy(osb_view(g, lv * 8, lv * 8 + 8),
                                      pview(pfin))

            def p_transposes(g, pfin, pT_dst):
                """pfin [128, 4bt x (32 probs | one)] -> pT_dst[0:33, b]."""
                for bti in range(4):
                    bt = 4 * g + bti
                    trp = ps_tr.tile([P_IN + 1, 128], BF16, tag="ptr")
                    nc.tensor.transpose(
                        trp, pfin[:, bti * P_HID:bti * P_HID + P_IN + 1],
                        ident_v)
                    nc.vector.tensor_copy(
                        pT_dst[:, bt * 128:(bt + 1) * 128], trp)

            def mid_h_mms(g):
                for q in range(4):
                    ps = ps_mm.tile([128, 512], F32, tag="mm")
                    nc.tensor.matmul(
                        ps, w1c_v[:, q, :],
                        prT_sb[:, g * 512:(g + 1) * 512],
                        start=True, stop=True)
                    lrelu_psum(hT_sb[:, q, g * 512:(g + 1) * 512], ps)

            def mid_logit_mms(g, ml):
                for bti in range(4):
                    bt = 4 * g + bti
                    nc.tensor.matmul(
                        ml[:, bti * 32:(bti + 1) * 32], ones_sb, b2cr_v,
                        start=True, stop=False, skip_group_check=True)
                    for q in range(4):
                        nc.tensor.matmul(
                            ml[:, bti * 32 + q * 8:bti * 32 + (q + 1) * 8],
                            hT_sb[:, q, bt * 128:(bt + 1) * 128],
                            w2c_v[:, q, :],
                            start=False, stop=True,
                            skip_group_check=True)

            def task_mms(g, yl):
                ps = ps_mm.tile([P_HID, 512], F32, tag="mm")
                nc.tensor.matmul(
                    ps, w1y_v, pmT_sb[:, g * 512:(g + 1) * 512],
                    start=True, stop=True)
                # DVE-only drain (copy + lrelu): keeps the ACT engine out
                # of the critical task chain in the exposed tail
                st = stage_pool.tile([P_HID, 512], BF16, tag="encst")
                nc.vector.tensor_copy(st, ps)
                lrelu(hyT_sb[0:P_HID, g * 512:(g + 1) * 512], st)
                for bti in range(4):
                    bt = 4 * g + bti
                    nc.tensor.matmul(
                        yl[:, bti * 4:(bti + 1) * 4],
                        hyT_sb[:, bt * 128:(bt + 1) * 128], w2y_v,
                        start=True, stop=True, skip_group_check=True)

            def store_cm(g):
                """Store concept prob cols [0:64) of a half (issued before
                the task tail so only the tiny y-col store is exposed)."""
                nc.sync.dma_start(
                    out=out_d.ap().rearrange("(t p) k -> p t k", p=128)
                    [:, 4 * g:4 * g + 4, 0:64],
                    in_=osb_gs[g].rearrange("p (b k) -> p b k", k=OUTW)
                    [:, :, 0:64])

            def y_tail(g, yl):
                e4 = tmp8_pool.tile([128, 16], F32, tag="e4")
                nc.scalar.activation(e4, yl, AF.Exp)
                s1 = tmp8_pool.tile([128, 4], F32, tag="s1")
                nc.vector.reduce_sum(
                    s1, e4.rearrange("p (b c) -> p b c", c=CARD), axis=AX.X)
                r1 = tmp8_pool.tile([128, 4], F32, tag="r1")
                nc.vector.reciprocal(r1, s1)
                nc.vector.tensor_tensor(
                    osb_view(g, 16, 17).squeeze(2),
                    e4.rearrange("p (b c) -> p b c", c=CARD),
                    r1.unsqueeze(2).broadcast_to([128, 4, CARD]),
                    op=ALU.mult)
                nc.sync.dma_start(
                    out=out_d.ap().rearrange("(t p) k -> p t k", p=128)
                    [:, 4 * g:4 * g + 4, 64:68],
                    in_=osb_gs[g].rearrange("p (b k) -> p b k", k=OUTW)
                    [:, :, 64:68])

            # ================= emission schedule =================
            lg0 = ps_lg.tile([128, 128], F32, tag="lg", name="lg0")
            vals_scorer_half(0, lg0)

            # h0 root softmax chain (DVE/ACT) runs under enc(h1) on the PE
            pfin0 = softmax_chain(0, lg0, 0)

            # encoder h1: x fully resident, kt-inner 2-bank rotation
            for ht in range(8):
                ps = ps_mm.tile([128, 512], F32, tag="mm")
                for j in range(KJ_IN):
                    mm_pair(ps, wenc_sb[:, j, :, ht * 128:(ht + 1) * 128],
                            xt_sb[:, j, :, 512:1024],
                            start=(j == 0), stop=(j == KJ_IN - 1))
                drain_enc(ht, ps, 1)

            p_transposes(0, pfin0, prT_sb)
            osb_store(0, pfin0, 0)
            mid_h_mms(0)
            ml0 = ps_lg.tile([128, 128], F32, tag="lg", name="ml0")
            mid_logit_mms(0, ml0)

            lg1 = ps_lg.tile([128, 128], F32, tag="lg", name="lg1")

            def h0_mid_tail():
                pf = softmax_chain(0, ml0, 1)
                p_transposes(0, pf, pmT_sb)
                osb_store(0, pf, 1)

            def h0_task():
                yl0 = ps_lg.tile([128, 16], F32, tag="lg", name="yl0")
                store_cm(0)
                task_mms(0, yl0)
                y_tail(0, yl0)

            vals_scorer_half(1, lg1, extra_pe={1: h0_mid_tail, 4: h0_task})

            # ---------------- h1 tail (end of kernel) ----------------
            # small warm bursts keep the HAM un-throttled across the
            # PE-idle softmax windows (transpose-mode does not count as
            # PE activity for the HAM)
            pfin1 = softmax_chain(1, lg1, 0)
            warm(8, "warm1")
            p_transposes(1, pfin1, prT_sb)
            warm(3, "warm1b")
            osb_store(1, pfin1, 0)
            mid_h_mms(1)
            ml1 = ps_lg.tile([128, 128], F32, tag="lg", name="ml1")
            mid_logit_mms(1, ml1)
            pf = softmax_chain(1, ml1, 1)
            warm(6, "warm2")
            p_transposes(1, pf, pmT_sb)
            warm(3, "warm2b")
            osb_store(1, pf, 1)
            store_cm(1)
            yl1 = ps_lg.tile([128, 16], F32, tag="lg", name="yl1")
            task_mms(1, yl1)
            y_tail(1, yl1)

    nc.compile()
    return nc


def _pairs(a, kj):
    """[256*kj, N] -> [128, kj, 2, N] DoubleRow pair layout, flattened."""
    n = a.shape[1]
    return np.ascontiguousarray(
        a.reshape(kj, 2, 128, n).transpose(2, 0, 1, 3).reshape(128, -1))


def prep_weights(inp):
    """Host-side reformatting of (replicated) weights to device layouts."""
    f32 = np.float32
    W_enc = np.asarray(inp["W_enc"], f32)
    Wv = np.asarray(inp["Wv"], f32)
    Ws = np.asarray(inp["Ws"], f32)
    W1c = np.asarray(inp["W1c"], f32)
    W2c = np.asarray(inp["W2c"], f32)
    W1y = np.asarray(inp["W1y"], f32)
    W2y = np.asarray(inp["W2y"], f32)
    b1c = np.asarray(inp["b1c"], f32)
    b1y = np.asarray(inp["b1y"], f32)
    b2y = np.asarray(inp["b2y"], f32)

    wenc = _pairs(W_enc * SW, KJ_IN).astype(MDT_NP)
    wv_all = Wv.transpose(1, 0, 2).reshape(D_H, N_ROOT * OV)
    wv = _pairs(wv_all * SV, KJ_H).astype(MDT_NP)

    # bf16 canvas
    smb = np.zeros((128, SMB_COLS), f32)
    smb[:, C_WS:C_WS + 64] = (
        Ws.transpose(1, 0, 2).reshape(OV, N_ROOT * CARD)  # [o, (r c)]
        .reshape(2, 128, 32).transpose(1, 0, 2).reshape(128, 64))
    w1c_flat = W1c.transpose(1, 0, 2).reshape(P_IN, 512)
    smb[0:P_IN, C_W1C:C_W1C + 512] = w1c_flat
    smb[P_IN, C_W1C:C_W1C + 512] = b1c.reshape(4, 2, 64).reshape(512)
    w2c_bp = np.zeros((2, 64, 4, 2, 4), f32)
    for q in range(4):
        for sdx in range(2):
            w2c_bp[sdx, :, q, sdx, :] = W2c[2 * q + sdx]
    smb[:, C_W2C:C_W2C + 32] = w2c_bp.reshape(128, 32)
    smb[0:P_IN, C_W1Y:C_W1Y + P_HID] = W1y
    smb[P_IN, C_W1Y:C_W1Y + P_HID] = b1y
    smb[0:P_HID, C_W2Y:C_W2Y + CARD] = W2y
    smb[P_HID, C_W2Y:C_W2Y + CARD] = b2y
    smb[0, C_BSR:C_BSR + 32] = np.asarray(inp["bs"], f32).reshape(32)
    smb[0, C_B2CR:C_B2CR + 32] = np.asarray(inp["b2c"], f32).reshape(32)
    smb[:, C_IDENT:C_IDENT + 128] = np.eye(128, dtype=f32)

    f32c = np.zeros((128, F32_COLS), f32)
    f32c[:, C_BENC:C_BENC + 8] = \
        np.asarray(inp["b_enc"], f32).reshape(8, 128).T * SH
    f32c[:, C_BV:C_BV + 16] = (np.asarray(inp["bv"], f32)
                               .reshape(N_ROOT, 2, 128)
                               .transpose(2, 0, 1).reshape(128, 16))
    f32c[:, C_IOTA:C_IOTA + 32] = np.tile(
        np.arange(CARD, dtype=f32), (128, N_ROOT))

    return {
        "wenc": wenc,
        "wv": wv,
        "smb": np.ascontiguousarray(smb, ml_dtypes.bfloat16),
        "f32c": np.ascontiguousarray(f32c),
    }


def make_in_maps(inp):
    wmap = prep_weights(inp)
    x = np.asarray(inp["x"], np.float32)
    lab = np.asarray(inp["c"], np.int32)
    msk = np.asarray(inp["intervention_index"], np.int32)
    in_maps = []
    for i in range(N_CORES):
        m = dict(wmap)
        xs = x[i * BSH:(i + 1) * BSH]                   # [1024, 2048]
        m["xt"] = _pairs(xs.T * SX, KJ_IN).astype(MDT_NP)
        lm = np.empty((128, 272), np.int32)
        lm[:, 0:136] = (lab[i * BSH:(i + 1) * BSH]
                        .reshape(8, 128, 17).transpose(1, 0, 2)
                        .reshape(128, 136))
        lm[:, 136:272] = (msk[i * BSH:(i + 1) * BSH]
                          .reshape(8, 128, 17).transpose(1, 0, 2)
                          .reshape(128, 136))
        m["lm"] = np.ascontiguousarray(lm)
        in_maps.append(m)
    return in_maps


_NC_CACHE = {}


def _get_nc():
    key = (USE_FP8, N_WARM)
    if key not in _NC_CACHE:
        _NC_CACHE[key] = build_program()
    return _NC_CACHE[key]


def kernel(**inputs):
    from concourse.bass_utils import run_bass_kernel_spmd

    nc = _get_nc()
    in_maps = make_in_maps(inputs)
    res = run_bass_kernel_spmd(nc, in_maps, list(range(N_CORES)))
    outs = [np.asarray(res.results[i]["out"], np.float32).reshape(BSH, 17, CARD)
            for i in range(N_CORES)]
    return np.concatenate(outs, axis=0)


# revision 28
# speedup vs baseline: 1.0833x; 1.0833x over previous
"""Trainium2 Bass kernel for nn_C2BM_30537217474758 (gnn_message_passing).

Concept-bottleneck model:
  x_enc = lrelu(x @ W_enc + b_enc)                         [B, 1024]
  vals  = lrelu(einsum('bi,rio->bro', x_enc, Wv) + bv)     [B, 8, 256]
  p_root = softmax(einsum('bro,roc->brc', vals, Ws) + bs)  [B, 8, 4]
  p_root = intervene(p_root, c[:, :8], ii[:, :8])
  h     = lrelu(einsum('bp,nph->bnh', p_root.flat, W1c) + b1c)
  p_mid = softmax(einsum('bnh,nhc->bnc', h, W2c) + b2c); intervene
  y     = softmax(lrelu(p_mid.flat @ W1y + b1y) @ W2y + b2y)
  out   = concat([p_root, p_mid, y[:, None]], axis=1)      [B, 17, 4]

Strategy: pure data-parallel over 8 NeuronCores (batch shard 1024/core),
weights replicated.  The two large GEMMs (encoder and value-embedding) run
as fp8e4m3 DoubleRow matmuls (256-row virtual contraction, ~1.5x bf16 PE
rate) with fp32 PSUM accumulation; inputs are pre-scaled on the host to
keep fp8 operands in the normal range and the scales are unwound in the
PSUM drains.  x is transposed AND pre-quantized on the host, so no
on-chip transposes of x are needed at all.  All leaky-relus run on the
DVE (max(x, .01x)); the ACT engine only ever evaluates Identity/Exp so
its function-table is loaded exactly once.  Small weights are packed into
three canvas tensors loaded in single DMAs before the big streams, and
x/W stream in interleaved 256KB chunks so the encoder starts ~4us in.
Dummy matmuls at the head keep the PE HAM un-throttled through the DMA
ramp.  Everything downstream of the big GEMMs (softmax, intervention,
propagator MLPs) is bf16/fp32 exactly like the reference.
"""

import os
import sys

try:
    import concourse  # noqa: F401
except ImportError:
    sys.path.insert(0, "/opt/trn_rl_repo")

import numpy as np
import ml_dtypes

import concourse.bacc as bacc
import concourse.tile as tile
from concourse import mybir

# ---------------- problem constants (hardcoded per contract) ----------------
B, D_IN, D_H = 8192, 2048, 1024
N_ROOT, N_MID, CARD, CHS = 8, 8, 4, 64
OV = CARD * CHS           # 256  value-embedding width per root
P_IN = N_ROOT * CARD      # 32
P_HID = 2 * P_IN          # 64
N_CORES = 8
BSH = B // N_CORES        # 1024 batch rows per core
OUTW = 17 * CARD          # 68 output cols per row
KJ_IN = D_IN // 256       # 8 DoubleRow k-chunks for encoder
KJ_H = D_H // 256         # 4 DoubleRow k-chunks for Wv

F32 = mybir.dt.float32
I32 = mybir.dt.int32
BF16 = mybir.dt.bfloat16
FP8 = mybir.dt.float8e4
AF = mybir.ActivationFunctionType
ALU = mybir.AluOpType
AX = mybir.AxisListType
DR = mybir.MatmulPerfMode.DoubleRow

USE_FP8 = os.environ.get("BASS_FP8", "1") != "0"
MDT = FP8 if USE_FP8 else BF16
MDT_NP = ml_dtypes.float8_e4m3 if USE_FP8 else ml_dtypes.bfloat16
LRELU_ALPHA = 0.01
N_WARM = int(os.environ.get("BASS_WARM", "28"))
# parametric_relu is a filler function resident in EVERY ACT table set
# (unlike leaky_relu), so fused Prelu drains cost zero table switches.
# CoreSim doesn't implement Prelu -> BASS_PRELU=0 swaps in the
# numerically-identical Identity + DVE max(x, .01x) pair for simulation.
USE_PRELU = os.environ.get("BASS_PRELU", "1") != "0"

# fp8 pre-scales (host) and their unwind factors (on-chip activation scale)
SX = 16.0 if USE_FP8 else 1.0    # x
SW = 512.0 if USE_FP8 else 1.0   # W_enc
SH = 32.0 if USE_FP8 else 1.0    # x_enc (re-quantized fp8 out of encoder)
SV = 512.0 if USE_FP8 else 1.0   # Wv
SCALE1 = SH / (SX * SW)          # psum1 -> SH * x_enc
SCALE2 = 1.0 / (SH * SV)         # psum2 -> vals (unscaled)

# bf16 canvas column layout
C_WS = 0            # [128, 2, 32]  scorer Ws
C_W1C = 64          # [33, 4, 128]  mid W1c pairs + b1c ones-row
C_W2C = 576         # [128, 4, 8]   mid W2c block-pairs
C_W1Y = 608         # [33, 64]      task W1y + b1y row
C_W2Y = 672         # [65, 4]       task W2y + b2y row
C_BSR = 676         # [1, 32]       root scorer bias row
C_B2CR = 708        # [1, 32]       mid bias row
C_IDENT = 740       # [128, 128]    identity for PE transposes
SMB_COLS = 868
# f32 canvas
C_BENC = 0          # [128, 8]   SH * b_enc, [p, ht]
C_BV = 8            # [128, 16]  bv, [p, (r, ot)]
C_IOTA = 24         # [128, 32]  tiled iota(4) for one-hot build
F32_COLS = 56


def build_program():
    """Emit the per-core Bass program (identical on all 8 cores)."""
    nc = bacc.Bacc("TRN2", target_bir_lowering=False, debug=False,
                   num_devices=N_CORES)

    # ------------- DRAM I/O -------------
    xt_d = nc.dram_tensor("xt", [128, KJ_IN * 2 * BSH], MDT,
                          kind="ExternalInput")
    wenc_d = nc.dram_tensor("wenc", [128, KJ_IN * 2 * D_H], MDT,
                            kind="ExternalInput")
    wv_d = nc.dram_tensor("wv", [128, KJ_H * 2 * N_ROOT * OV], MDT,
                          kind="ExternalInput")
    smb_d = nc.dram_tensor("smb", [128, SMB_COLS], BF16, kind="ExternalInput")
    f32c_d = nc.dram_tensor("f32c", [128, F32_COLS], F32, kind="ExternalInput")
    lm_d = nc.dram_tensor("lm", [128, 2 * 8 * 17], I32, kind="ExternalInput")
    out_d = nc.dram_tensor("out", [BSH, OUTW], F32, kind="ExternalOutput")

    with tile.TileContext(nc) as tc:
        with (
            tc.tile_pool(name="persist", bufs=1) as persist,
            tc.tile_pool(name="vals", bufs=2) as vals_pool,
            tc.tile_pool(name="stage", bufs=3) as stage_pool,
            tc.tile_pool(name="tmp32", bufs=4) as tmp32_pool,
            tc.tile_pool(name="tmp8", bufs=6) as tmp8_pool,
            tc.tile_pool(name="ps_mm", bufs=4, space="PSUM") as ps_mm,
            tc.tile_pool(name="ps_lg", bufs=2, space="PSUM") as ps_lg,
            tc.tile_pool(name="ps_tr", bufs=2, space="PSUM") as ps_tr,
        ):
            # ---------------- persistent SBUF ----------------
            smb_sb = persist.tile([128, SMB_COLS], BF16)
            f32c_sb = persist.tile([128, F32_COLS], F32)
            lm_sb = persist.tile([128, 2 * 8 * 17], I32)
            xt_sb = persist.tile([128, KJ_IN, 2, BSH], MDT)
            wenc_sb = persist.tile([128, KJ_IN, 2, D_H], MDT)
            wv_sb = persist.tile([128, KJ_H, 2, N_ROOT * OV], MDT)
            xenc_sb = persist.tile([128, KJ_H, 2, BSH], MDT)
            prT_sb = persist.tile([P_IN + 1, BSH], BF16)
            pmT_sb = persist.tile([P_IN + 1, BSH], BF16)
            hyT_sb = persist.tile([P_HID + 1, BSH], BF16)
            hT_sb = persist.tile([128, 4, BSH], BF16)
            ones_sb = persist.tile([1, 128], BF16)
            spam_sb = persist.tile([128, 512], BF16)
            dume_sb = persist.tile([128, 8], F32)
            osb_gs = [persist.tile([128, 4 * OUTW], F32, name=f"osbg{i}")
                      for i in range(2)]

            ws_v = smb_sb[:, C_WS:C_WS + 64].rearrange(
                "p (k c) -> p k c", c=32)
            w1c_v = smb_sb[0:P_IN + 1, C_W1C:C_W1C + 512].rearrange(
                "p (q m) -> p q m", m=128)
            w2c_v = smb_sb[:, C_W2C:C_W2C + 32].rearrange(
                "p (q c) -> p q c", c=8)
            w1y_v = smb_sb[0:P_IN + 1, C_W1Y:C_W1Y + P_HID]
            w2y_v = smb_sb[0:P_HID + 1, C_W2Y:C_W2Y + CARD]
            bsr_v = smb_sb[0:1, C_BSR:C_BSR + 32]
            b2cr_v = smb_sb[0:1, C_B2CR:C_B2CR + 32]
            ident_v = smb_sb[:, C_IDENT:C_IDENT + 128]
            benc_v = f32c_sb[:, C_BENC:C_BENC + 8]
            bv_v = f32c_sb[:, C_BV:C_BV + 16]
            iota_v = f32c_sb[:, C_IOTA:C_IOTA + 32]
            lab_v = lm_sb[:, 0:136].rearrange("p (t k) -> p t k", k=17)
            msk_v = lm_sb[:, 136:272].rearrange("p (t k) -> p t k", k=17)

            # ------------- prologue: memsets + ACT table pin -------------
            nc.vector.memset(ones_sb, 1.0)
            nc.vector.memset(spam_sb, 0.0)
            nc.vector.memset(dume_sb, 0.0)
            nc.vector.memset(hyT_sb[P_HID:P_HID + 1, :], 1.0)
            dume2 = persist.tile([128, 8], F32, name="dume2")
            # first ACT instruction is an Exp -> its table set stays
            # resident for the whole kernel (Identity rides along free)
            nc.scalar.activation(dume2, dume_sb, AF.Exp)

            # ------------- DMA schedule: smalls first, then streams ------
            nc.sync.dma_start(out=smb_sb, in_=smb_d.ap())
            nc.sync.dma_start(out=f32c_sb, in_=f32c_d.ap())
            nc.sync.dma_start(out=lm_sb, in_=lm_d.ap())
            wenc_r = wenc_d.ap().rearrange("p (j i k) -> p j i k",
                                           j=KJ_IN, i=2)
            xt_r = xt_d.ap().rearrange("p (j i k) -> p j i k", j=KJ_IN, i=2)
            wv_r = wv_d.ap().rearrange("p (j i k) -> p j i k", j=KJ_H, i=2)
            for j in range(KJ_IN):
                nc.sync.dma_start(out=wenc_sb[:, j], in_=wenc_r[:, j])
                nc.sync.dma_start(out=xt_sb[:, j], in_=xt_r[:, j])
            # wv in 4 column chunks (2 roots each) so vals r0/r1 can start
            # before the whole tensor lands
            for c in range(4):
                nc.sync.dma_start(out=wv_sb[:, :, :, 512 * c:512 * (c + 1)],
                                  in_=wv_r[:, :, :, 512 * c:512 * (c + 1)])

            # ------------- PE warm-up spam (during DMA ramp) -------------
            def warm(n, name):
                ps = ps_mm.tile([128, 128], F32, tag="mm", name=name)
                for _ in range(n):
                    nc.tensor.matmul(ps, spam_sb[:, 0:128],
                                     spam_sb[:, 0:128],
                                     start=True, stop=True,
                                     skip_group_check=True)

            warm(N_WARM, "warm0")

            # ------------- helpers -------------
            def mm_pair(ps, w4, x4, start, stop):
                """One 256-deep contraction step: DoubleRow (fp8) or two
                bf16 matmuls. w4/x4 are [128, 2, M/N] slices."""
                if USE_FP8:
                    nc.tensor.matmul(ps, w4, x4, start=start, stop=stop,
                                     perf_mode=DR)
                else:
                    nc.tensor.matmul(ps, w4[:, 0], x4[:, 0],
                                     start=start, stop=False)
                    nc.tensor.matmul(ps, w4[:, 1], x4[:, 1],
                                     start=False, stop=stop)

            def lrelu(out, in_):
                nc.vector.scalar_tensor_tensor(out, in_, LRELU_ALPHA, in_,
                                               op0=ALU.mult, op1=ALU.max)

            def act_lrelu(out, ps, bias=0.0, scale=1.0, nrows=128):
                """lrelu(scale*ps + bias) drain: fused ACT Prelu on HW, or
                ACT Identity + DVE max-pair for CoreSim."""
                if USE_PRELU:
                    nc.scalar.activation(out, ps, AF.Prelu, bias=bias,
                                         scale=scale, alpha=LRELU_ALPHA)
                else:
                    st = stage_pool.tile([nrows, 512], BF16, tag="encst")
                    nc.scalar.activation(st, ps, AF.Identity, bias=bias,
                                         scale=scale)
                    lrelu(out, st)

            def drain_enc(ht, ps, g):
                act_lrelu(xenc_sb[:, ht // 2, ht % 2, g * 512:(g + 1) * 512],
                          ps, bias=benc_v[:, ht:ht + 1], scale=SCALE1)

            # ------------- encoder h0: kt-outer, 6+2 banks -------
            # Pass A uses 4 ps_mm banks plus the (still idle) ps_lg banks
            # so six output tiles track the x/W DMA chunk cadence; pass B
            # (2 tiles, ps_tr banks) runs while pass A drains.
            for hts, pools in ([0, 1, 2, 3, 4, 5], (ps_mm, ps_mm, ps_mm,
                                                    ps_mm, ps_lg, ps_lg)), \
                              ([6, 7], (ps_tr, ps_tr)):
                tagof = {id(ps_mm): "mm", id(ps_lg): "lg", id(ps_tr): "ptr"}
                pss = [pool.tile([128, 512], F32, tag=tagof[id(pool)],
                                 name=f"encp{ht}")
                       for ht, pool in zip(hts, pools)]
                for j in range(KJ_IN):
                    for i, ht in enumerate(hts):
                        mm_pair(pss[i],
                                wenc_sb[:, j, :, ht * 128:(ht + 1) * 128],
                                xt_sb[:, j, :, 0:512],
                                start=(j == 0), stop=(j == KJ_IN - 1))
                for i, ht in enumerate(hts):
                    drain_enc(ht, pss[i], 0)

            # ------------- intervention one-hots / masks (DVE) -----------
            oh_t, m_t = {}, {}

            def pview(t):
                return (t.rearrange("p (b k) -> p b k", k=P_HID)[:, :, 0:32]
                        .rearrange("p b (g c) -> p b g c", c=CARD))

            def make_ohm(g, lv):
                labf = tmp8_pool.tile([128, 32], F32, tag="labf")
                nc.vector.tensor_copy(
                    labf.rearrange("p (b g) -> p b g", b=4),
                    lab_v[:, 4 * g:4 * g + 4, lv * 8:lv * 8 + 8])
                oh = persist.tile([128, 256], BF16, name=f"oh{g}{lv}")
                nc.vector.tensor_tensor(
                    pview(oh),
                    labf.rearrange("p (b g) -> p b g", b=4)
                    .unsqueeze(3).broadcast_to([128, 4, 8, CARD]),
                    iota_v.rearrange("p (g c) -> p g c", c=CARD)
                    .unsqueeze(1).broadcast_to([128, 4, 8, CARD]),
                    op=ALU.is_equal)
                m = persist.tile([128, 256], I32, name=f"m{g}{lv}")
                nc.vector.tensor_copy(
                    pview(m),
                    msk_v[:, 4 * g:4 * g + 4, lv * 8:lv * 8 + 8]
                    .unsqueeze(3).broadcast_to([128, 4, 8, CARD]))
                oh_t[(g, lv)] = oh
                m_t[(g, lv)] = m

            for g in range(2):
                for lv in range(2):
                    make_ohm(g, lv)

            # ------------- per-root value GEMM + scorer (one half) -------
            def vals_scorer_half(g, lg, extra_pe=None):
                for bti in range(4):
                    nc.tensor.matmul(
                        lg[:, bti * 32:(bti + 1) * 32], ones_sb, bsr_v,
                        start=True, stop=False, skip_group_check=True)
                for r in range(N_ROOT):
                    vals_sb = vals_pool.tile([128, 2, 512], BF16, tag="vals")
                    for ot in range(2):
                        ps = ps_mm.tile([128, 512], F32, tag="mm")
                        for jh in range(KJ_H):
                            mm_pair(ps,
                                    wv_sb[:, jh, :,
                                          r * OV + ot * 128:
                                          r * OV + (ot + 1) * 128],
                                    xenc_sb[:, jh, :, g * 512:(g + 1) * 512],
                                    start=(jh == 0), stop=(jh == KJ_H - 1))
                        act_lrelu(vals_sb[:, ot], ps,
                                  bias=bv_v[:, 2 * r + ot:2 * r + ot + 1],
                                  scale=SCALE2)
                    for bti in range(4):
                        dst = lg[:, bti * 32 + r * 4:bti * 32 + r * 4 + 4]
                        for kt in range(2):
                            nc.tensor.matmul(
                                dst, vals_sb[:, kt, bti * 128:(bti + 1) * 128],
                                ws_v[:, kt, r * 4:(r + 1) * 4],
                                start=False, stop=(kt == 1),
                                skip_group_check=True)
                    if extra_pe and r in extra_pe:
                        extra_pe[r]()

            # ---------------- tail stages for one half ----------------
            def softmax_chain(g, lg, lv):
                """softmax + intervention on [128, 4bt x 32] logits ->
                pfin [128, 4bt x (32 probs | one | pad)] bf16."""
                e = tmp32_pool.tile([128, 128], F32, tag="e")
                nc.scalar.activation(e, lg, AF.Exp)
                s = tmp8_pool.tile([128, 32], F32, tag="s")
                nc.vector.reduce_sum(s, e.rearrange("p (x c) -> p x c",
                                                    c=CARD), axis=AX.X)
                rcp = tmp8_pool.tile([128, 32], F32, tag="rcp")
                nc.vector.reciprocal(rcp, s)
                pfin = tmp32_pool.tile([128, 256], BF16, tag="pfin")
                nc.vector.memset(
                    pfin.rearrange("p (b k) -> p b k", k=P_HID)[:, :, 32:33],
                    1.0)
                nc.vector.tensor_tensor(
                    pview(pfin),
                    e.rearrange("p (b g c) -> p b g c", b=4, c=CARD),
                    rcp.rearrange("p (b g) -> p b g", b=4)
                    .unsqueeze(3).broadcast_to([128, 4, 8, CARD]),
                    op=ALU.mult)
                nc.vector.copy_predicated(pview(pfin), pview(m_t[(g, lv)]),
                                          pview(oh_t[(g, lv)]))
                return pfin

            def osb_view(g, lo, hi):
                return (osb_gs[g].rearrange("p (b k) -> p b k", k=OUTW)
                        [:, :, lo * 4:hi * 4]
                        .rearrange("p b (gg c) -> p b gg c", c=CARD))

            def osb_store(g, pfin, lv):
                nc.vector.tensor_copy(osb_view(g, lv * 8, lv * 8 + 8),
                                      pview(pfin))

            def p_transposes(g, pfin, pT_dst):
                """pfin [128, 4bt x (32 probs | one)] -> pT_dst[0:33, b]."""
                for bti in range(4):
                    bt = 4 * g + bti
                    trp = ps_tr.tile([P_IN + 1, 128], BF16, tag="ptr")
                    nc.tensor.transpose(
                        trp, pfin[:, bti * P_HID:bti * P_HID + P_IN + 1],
                        ident_v)
                    nc.vector.tensor_copy(
                        pT_dst[:, bt * 128:(bt + 1) * 128], trp)

            def mid_h_mms(g):
                for q in range(4):
                    ps = ps_mm.tile([128, 512], F32, tag="mm")
                    nc.tensor.matmul(
                        ps, w1c_v[:, q, :],
                        prT_sb[:, g * 512:(g + 1) * 512],
                        start=True, stop=True)
                    act_lrelu(hT_sb[:, q, g * 512:(g + 1) * 512], ps)

            def mid_logit_mms(g, ml):
                for bti in range(4):
                    bt = 4 * g + bti
                    nc.tensor.matmul(
                        ml[:, bti * 32:(bti + 1) * 32], ones_sb, b2cr_v,
                        start=True, stop=False, skip_group_check=True)
                    for q in range(4):
                        nc.tensor.matmul(
                            ml[:, bti * 32 + q * 8:bti * 32 + (q + 1) * 8],
                            hT_sb[:, q, bt * 128:(bt + 1) * 128],
                            w2c_v[:, q, :],
                            start=False, stop=True,
                            skip_group_check=True)

            def task_mms(g, yl):
                ps = ps_mm.tile([P_HID, 512], F32, tag="mm")
                nc.tensor.matmul(
                    ps, w1y_v, pmT_sb[:, g * 512:(g + 1) * 512],
                    start=True, stop=True)
                act_lrelu(hyT_sb[0:P_HID, g * 512:(g + 1) * 512], ps,
                          nrows=P_HID)
                for bti in range(4):
                    bt = 4 * g + bti
                    nc.tensor.matmul(
                        yl[:, bti * 4:(bti + 1) * 4],
                        hyT_sb[:, bt * 128:(bt + 1) * 128], w2y_v,
                        start=True, stop=True, skip_group_check=True)

            def store_cm(g):
                """Store concept prob cols [0:64) of a half (issued before
                the task tail so only the tiny y-col store is exposed)."""
                nc.sync.dma_start(
                    out=out_d.ap().rearrange("(t p) k -> p t k", p=128)
                    [:, 4 * g:4 * g + 4, 0:64],
                    in_=osb_gs[g].rearrange("p (b k) -> p b k", k=OUTW)
                    [:, :, 0:64])

            def y_tail(g, yl):
                e4 = tmp8_pool.tile([128, 16], F32, tag="e4")
                nc.scalar.activation(e4, yl, AF.Exp)
                s1 = tmp8_pool.tile([128, 4], F32, tag="s1")
                nc.vector.reduce_sum(
                    s1, e4.rearrange("p (b c) -> p b c", c=CARD), axis=AX.X)
                r1 = tmp8_pool.tile([128, 4], F32, tag="r1")
                nc.vector.reciprocal(r1, s1)
                nc.vector.tensor_tensor(
                    osb_view(g, 16, 17).squeeze(2),
                    e4.rearrange("p (b c) -> p b c", c=CARD),
                    r1.unsqueeze(2).broadcast_to([128, 4, CARD]),
                    op=ALU.mult)
                nc.sync.dma_start(
                    out=out_d.ap().rearrange("(t p) k -> p t k", p=128)
                    [:, 4 * g:4 * g + 4, 64:68],
                    in_=osb_gs[g].rearrange("p (b k) -> p b k", k=OUTW)
                    [:, :, 64:68])

            # ================= emission schedule =================
            lg0 = ps_lg.tile([128, 128], F32, tag="lg", name="lg0")
            vals_scorer_half(0, lg0)

            # h0 root softmax chain (DVE/ACT) runs under enc(h1) on the PE
            pfin0 = softmax_chain(0, lg0, 0)

            # encoder h1: x fully resident, kt-inner 2-bank rotation
            for ht in range(8):
                ps = ps_mm.tile([128, 512], F32, tag="mm")
                for j in range(KJ_IN):
                    mm_pair(ps, wenc_sb[:, j, :, ht * 128:(ht + 1) * 128],
                            xt_sb[:, j, :, 512:1024],
                            start=(j == 0), stop=(j == KJ_IN - 1))
                drain_enc(ht, ps, 1)

            p_transposes(0, pfin0, prT_sb)
            osb_store(0, pfin0, 0)
            mid_h_mms(0)
            ml0 = ps_lg.tile([128, 128], F32, tag="lg", name="ml0")
            mid_logit_mms(0, ml0)

            lg1 = ps_lg.tile([128, 128], F32, tag="lg", name="lg1")

            def h0_mid_tail():
                pf = softmax_chain(0, ml0, 1)
                p_transposes(0, pf, pmT_sb)
                osb_store(0, pf, 1)

            def h0_task():
                yl0 = ps_lg.tile([128, 16], F32, tag="lg", name="yl0")
                store_cm(0)
                task_mms(0, yl0)
                y_tail(0, yl0)

            vals_scorer_half(1, lg1, extra_pe={1: h0_mid_tail, 4: h0_task})

            # ---------------- h1 tail (end of kernel) ----------------
            # small warm bursts keep the HAM un-throttled across the
            # PE-idle softmax windows (transpose-mode does not count as
            # PE activity for the HAM)
            pfin1 = softmax_chain(1, lg1, 0)
            p_transposes(1, pfin1, prT_sb)
            osb_store(1, pfin1, 0)
            mid_h_mms(1)
            ml1 = ps_lg.tile([128, 128], F32, tag="lg", name="ml1")
            mid_logit_mms(1, ml1)
            pf = softmax_chain(1, ml1, 1)
            p_transposes(1, pf, pmT_sb)
            osb_store(1, pf, 1)
            store_cm(1)
            yl1 = ps_lg.tile([128, 16], F32, tag="lg", name="yl1")
            task_mms(1, yl1)
            y_tail(1, yl1)

    nc.compile()
    return nc


def _pairs(a, kj):
    """[256*kj, N] -> [128, kj, 2, N] DoubleRow pair layout, flattened."""
    n = a.shape[1]
    return np.ascontiguousarray(
        a.reshape(kj, 2, 128, n).transpose(2, 0, 1, 3).reshape(128, -1))


def prep_weights(inp):
    """Host-side reformatting of (replicated) weights to device layouts."""
    f32 = np.float32
    W_enc = np.asarray(inp["W_enc"], f32)
    Wv = np.asarray(inp["Wv"], f32)
    Ws = np.asarray(inp["Ws"], f32)
    W1c = np.asarray(inp["W1c"], f32)
    W2c = np.asarray(inp["W2c"], f32)
    W1y = np.asarray(inp["W1y"], f32)
    W2y = np.asarray(inp["W2y"], f32)
    b1c = np.asarray(inp["b1c"], f32)
    b1y = np.asarray(inp["b1y"], f32)
    b2y = np.asarray(inp["b2y"], f32)

    wenc = _pairs(W_enc * SW, KJ_IN).astype(MDT_NP)
    wv_all = Wv.transpose(1, 0, 2).reshape(D_H, N_ROOT * OV)
    wv = _pairs(wv_all * SV, KJ_H).astype(MDT_NP)

    # bf16 canvas
    smb = np.zeros((128, SMB_COLS), f32)
    smb[:, C_WS:C_WS + 64] = (
        Ws.transpose(1, 0, 2).reshape(OV, N_ROOT * CARD)  # [o, (r c)]
        .reshape(2, 128, 32).transpose(1, 0, 2).reshape(128, 64))
    w1c_flat = W1c.transpose(1, 0, 2).reshape(P_IN, 512)
    smb[0:P_IN, C_W1C:C_W1C + 512] = w1c_flat
    smb[P_IN, C_W1C:C_W1C + 512] = b1c.reshape(4, 2, 64).reshape(512)
    w2c_bp = np.zeros((2, 64, 4, 2, 4), f32)
    for q in range(4):
        for sdx in range(2):
            w2c_bp[sdx, :, q, sdx, :] = W2c[2 * q + sdx]
    smb[:, C_W2C:C_W2C + 32] = w2c_bp.reshape(128, 32)
    smb[0:P_IN, C_W1Y:C_W1Y + P_HID] = W1y
    smb[P_IN, C_W1Y:C_W1Y + P_HID] = b1y
    smb[0:P_HID, C_W2Y:C_W2Y + CARD] = W2y
    smb[P_HID, C_W2Y:C_W2Y + CARD] = b2y
    smb[0, C_BSR:C_BSR + 32] = np.asarray(inp["bs"], f32).reshape(32)
    smb[0, C_B2CR:C_B2CR + 32] = np.asarray(inp["b2c"], f32).reshape(32)
    smb[:, C_IDENT:C_IDENT + 128] = np.eye(128, dtype=f32)

    f32c = np.zeros((128, F32_COLS), f32)
    f32c[:, C_BENC:C_BENC + 8] = \
        np.asarray(inp["b_enc"], f32).reshape(8, 128).T * SH
    f32c[:, C_BV:C_BV + 16] = (np.asarray(inp["bv"], f32)
                               .reshape(N_ROOT, 2, 128)
                               .transpose(2, 0, 1).reshape(128, 16))
    f32c[:, C_IOTA:C_IOTA + 32] = np.tile(
        np.arange(CARD, dtype=f32), (128, N_ROOT))

    return {
        "wenc": wenc,
        "wv": wv,
        "smb": np.ascontiguousarray(smb, ml_dtypes.bfloat16),
        "f32c": np.ascontiguousarray(f32c),
    }


def make_in_maps(inp):
    wmap = prep_weights(inp)
    x = np.asarray(inp["x"], np.float32)
    lab = np.asarray(inp["c"], np.int32)
    msk = np.asarray(inp["intervention_index"], np.int32)
    in_maps = []
    for i in range(N_CORES):
        m = dict(wmap)
        xs = x[i * BSH:(i + 1) * BSH]                   # [1024, 2048]
        m["xt"] = _pairs(xs.T * SX, KJ_IN).astype(MDT_NP)
        lm = np.empty((128, 272), np.int32)
        lm[:, 0:136] = (lab[i * BSH:(i + 1) * BSH]
                        .reshape(8, 128, 17).transpose(1, 0, 2)
                        .reshape(128, 136))
        lm[:, 136:272] = (msk[i * BSH:(i + 1) * BSH]
                          .reshape(8, 128, 17).transpose(1, 0, 2)
                          .reshape(128, 136))
        m["lm"] = np.ascontiguousarray(lm)
        in_maps.append(m)
    return in_maps


_NC_CACHE = {}


def _get_nc():
    key = (USE_FP8, N_WARM, USE_PRELU)
    if key not in _NC_CACHE:
        _NC_CACHE[key] = build_program()
    return _NC_CACHE[key]


def kernel(**inputs):
    from concourse.bass_utils import run_bass_kernel_spmd

    nc = _get_nc()
    in_maps = make_in_maps(inputs)
    res = run_bass_kernel_spmd(nc, in_maps, list(range(N_CORES)))
    outs = [np.asarray(res.results[i]["out"], np.float32).reshape(BSH, 17, CARD)
            for i in range(N_CORES)]
    return np.concatenate(outs, axis=0)
